# revision 10
# baseline (speedup 1.0000x reference)
"""Trainium2 Bass kernel for nn_AttHeteroRGCNLayer (GAT-style hetero GNN layer).

Strategy (8 NeuronCores, SPMD):
  - dst-sharded edge phase: dsts are degree-snake-dealt to cores; every edge of
    a dst lives on one core, so segment softmax is core-local (no collectives
    for softmax statistics).
  - per core, edges are split by src range (int16 gather-index limit) into two
    independent structures; each packs dsts by degree into 128-slot blocks.
    A (block, slot, tile) grid assigns edge t of dst-slot p to lane p of tile
    t; the scatter-sum becomes diag(E) matmuls accumulating in PSUM.
  - projections are data-parallel GEMMs + AllGather of a bf16 node table
    [N x 384]: cols 0..255 = bf16(x @ W), f32 (x @ (W a1)) punned at 256-7.
  - per-slot s_dst comes from a small GEMV over host-permuted x columns.
  - host merges the two halves' unnormalized (h|z) grids, divides, adds bias.
"""

import os
import sys
import numpy as np

for _p in ("/opt/trn_rl_repo", "/root/.axon_site/_ro/trn_rl_repo"):
    if os.path.isdir(_p) and _p not in sys.path:
        sys.path.append(_p)

import ml_dtypes  # noqa: E402

BF16 = ml_dtypes.bfloat16
D = 256
NCORES = 8
TCOLS = 384            # table row = 768B (256 msg bf16 | s_src f32 | junk)
SCOL_F32 = 128         # f32 column of s_src in the 192-col f32 view of a row
TPC = 8                # tiles per dma_gather call (1024 idx: SWDGE ring limit)
CT = 40                # tiles per SBUF gather chunk (must be multiple of TPC)


def _default_cfg(n_nodes):
    split = 32768 if n_nodes > 32768 else n_nodes
    return {
        "N": n_nodes,
        "SPLIT": split,                      # half0: src < SPLIT
        "HI_BASE": max(0, n_nodes - 32768),  # half1 idx = src - HI_BASE
    }


# ----------------------------------------------------------------- host prep
def _build_etype(src, dst, cfg):
    N = cfg["N"]
    SPLIT = cfg["SPLIT"]
    deg = np.bincount(dst, minlength=N)
    order = np.argsort(-deg, kind="stable")
    core_of = np.empty(N, np.int32)
    fwd = np.arange(NCORES)
    rev = fwd[::-1]
    for i in range(0, N, 2 * NCORES):
        blk = order[i:i + NCORES]
        core_of[blk] = fwd[:len(blk)]
        blk = order[i + NCORES:i + 2 * NCORES]
        core_of[blk] = rev[:len(blk)]

    ecore = core_of[dst]
    half = (src >= SPLIT).astype(np.int8)

    percore = []
    BB = [1, 1]
    for c in range(NCORES):
        cdsts = np.where(core_of == c)[0]
        ent = {}
        for h in (0, 1):
            m = (ecore == c) & (half == h)
            hsrc, hdst = src[m], dst[m]
            hdeg = np.bincount(hdst, minlength=N)[cdsts]
            oo = np.argsort(-hdeg, kind="stable")
            ent[f"slots{h}"] = cdsts[oo]
            ent[f"sdeg{h}"] = hdeg[oo]
            ent[f"src{h}"] = hsrc
            ent[f"dst{h}"] = hdst
            nb = max(1, int(np.ceil(max(1, int((hdeg > 0).sum())) / 128)))
            BB[h] = max(BB[h], nb)
        percore.append(ent)

    nt = [np.ones(BB[0], np.int64), np.ones(BB[1], np.int64)]
    for c in range(NCORES):
        for h in (0, 1):
            sdeg = percore[c][f"sdeg{h}"]
            for b in range(BB[h]):
                blk = sdeg[b * 128:(b + 1) * 128]
                if len(blk) and blk.max() > nt[h][b]:
                    nt[h][b] = int(blk.max())
    T = [int(np.ceil(int(nt[h].sum()) / TPC) * TPC) for h in (0, 1)]
    return {"BB": BB, "nt": nt, "T": T, "percore": percore}


def _core_arrays(st, c, cfg):
    HI_BASE = cfg["HI_BASE"]
    per = st["percore"][c]
    res = {}
    for h in (0, 1):
        BBh, nth, T = st["BB"][h], st["nt"][h], st["T"][h]
        slots, sdeg = per[f"slots{h}"], per[f"sdeg{h}"]
        hsrc, hdst = per[f"src{h}"], per[f"dst{h}"]
        o = np.argsort(hdst, kind="stable")
        s_sorted, d_sorted = hsrc[o], hdst[o]
        uq, first = np.unique(d_sorted, return_index=True)
        start_of = dict(zip(uq.tolist(), first.tolist()))

        idx = np.zeros((T, 128), np.int32)
        mask = np.zeros((128, T), np.float32)       # (lane, tile)
        grid_nodes = np.full((BBh, 128), -1, np.int64)
        t0 = 0
        for b in range(BBh):
            bs = slots[b * 128:(b + 1) * 128]
            bd = sdeg[b * 128:(b + 1) * 128]
            ntb = int(nth[b])
            for p in range(len(bs)):
                dnode = int(bs[p])
                dg = int(bd[p])
                grid_nodes[b, p] = dnode
                if dg:
                    f = start_of[dnode]
                    v = s_sorted[f:f + dg]
                    idx[t0:t0 + dg, p] = v if h == 0 else v - HI_BASE
                    mask[p, t0:t0 + dg] = 1.0
            t0 += ntb
        res[f"idx{h}"] = idx
        res[f"mask{h}"] = mask
        res[f"grid{h}"] = grid_nodes
    return res


def _wrap_idx(idx_tl):
    """(tile, lane) int32 -> dma_gather wrapped int16 [128, n/16] (x8 groups)."""
    flat = idx_tl.reshape(-1)
    n = len(flat)
    a = np.zeros((16, n // 16), np.int16)
    a[np.arange(n) % 16, np.arange(n) // 16] = flat.astype(np.int16)
    return np.ascontiguousarray(np.tile(a, (8, 1)))


def _tile_xT(x_bf, ntiles):
    """x [M, D] bf16 -> tile-major [ntiles, D, 128] (zero padded)."""
    out = np.zeros((ntiles, D, 128), BF16)
    for t in range(ntiles):
        rows = x_bf[t * 128:(t + 1) * 128]
        if rows.shape[0]:
            out[t, :, :rows.shape[0]] = rows.T
    return out


def _perm_xT(x_bf, grids):
    """Permuted x columns for the s_dst GEMV: [BBtot, D, 128]."""
    mats = []
    for grid in grids:
        for b in range(grid.shape[0]):
            m = np.zeros((D, 128), BF16)
            gd = grid[b]
            valid = gd >= 0
            if valid.any():
                m[:, valid] = x_bf[gd[valid]].T
            mats.append(m)
    return np.stack(mats)


# ------------------------------------------------------------- device build
def _build_nc(gc):
    import concourse.bass as bass  # noqa: F401
    import concourse.mybir as mybir
    from concourse import bacc
    from concourse.tile import TileContext

    DT = mybir.dt
    Alu = mybir.AluOpType
    ActF = mybir.ActivationFunctionType
    N = gc["N"]
    SPLIT = gc["SPLIT"]
    HI_BASE = gc["HI_BASE"]
    SL_ROWS = gc["slice_rows"]
    NSLICE = gc["nslice_tiles"]
    ET = gc["etypes"]

    PH = set(os.environ.get("ATH_KERNEL_PHASES", "gemm,ag,grid,edge").split(","))
    nc = bacc.Bacc("TRN2", target_bir_lowering=False, debug=False,
                   num_devices=NCORES)
    ext = {}

    def din(name, shape, dt):
        ext[name] = nc.dram_tensor(name, shape, dt, kind="ExternalInput").ap()
        return ext[name]

    def dout(name, shape, dt):
        ext[name] = nc.dram_tensor(name, shape, dt, kind="ExternalOutput").ap()
        return ext[name]

    ident_e = din("ident", [128, 128], DT.bfloat16)
    for e in ET:
        n = e["name"]
        din(f"xT_{n}", [NSLICE, D, 128], DT.bfloat16)
        din(f"W_{n}", [D, D], DT.float32)
        din(f"a1r_{n}", [128, D], DT.float32)
        din(f"a2r_{n}", [128, D], DT.float32)
        din(f"br_{n}", [128, D], DT.float32)
        BBtot = e["BB"][0] + e["BB"][1]
        din(f"xperm_{n}", [BBtot, D, 128], DT.bfloat16)
        for h in (0, 1):
            din(f"idx{h}_{n}", [128, e["T"][h] * 8], DT.int16)
            din(f"mask{h}_{n}", [128, e["T"][h]], DT.float32)
            dout(f"hz{h}_{n}", [e["BB"][h], 128, 257], DT.float32)
        e["tbl"] = nc.dram_tensor(f"tbl_{n}", [N, TCOLS], DT.bfloat16,
                                  addr_space="Shared")
        e["loc"] = nc.dram_tensor(f"loc_{n}", [SL_ROWS, TCOLS], DT.bfloat16)

    with TileContext(nc) as tc:
        with tc.tile_pool(name="const", bufs=1) as cp, \
             tc.tile_pool(name="work", bufs=2) as wp, \
             tc.tile_pool(name="gbuf", bufs=int(os.environ.get("ATH_GBUFS", "3"))) as gp, \
             tc.tile_pool(name="psA", bufs=2, space="PSUM") as psA, \
             tc.tile_pool(name="psB", bufs=2, space="PSUM") as psB, \
             tc.tile_pool(name="psE", bufs=int(os.environ.get("ATH_PSE", "4")), space="PSUM") as psE:

            ident_sb = cp.tile([128, 128], DT.bfloat16, tag="ident")
            nc.sync.dma_start(out=ident_sb[:], in_=ident_e[:])

            # -------- per-etype weight prep (W, w1 hi/lo, rhs tiles) -------
            for e in ET:
                n = e["name"]
                W_sb = cp.tile([128, 2, D], DT.float32, tag=f"W_{n}")
                nc.sync.dma_start(out=W_sb[:, 0, :], in_=ext[f"W_{n}"][0:128, :])
                nc.sync.dma_start(out=W_sb[:, 1, :], in_=ext[f"W_{n}"][128:256, :])
                a1r = cp.tile([128, D], DT.float32, tag=f"a1r_{n}")
                nc.sync.dma_start(out=a1r[:], in_=ext[f"a1r_{n}"][:])
                a2r = cp.tile([128, D], DT.float32, tag=f"a2r_{n}")
                nc.sync.dma_start(out=a2r[:], in_=ext[f"a2r_{n}"][:])
                br = cp.tile([128, D], DT.float32, tag=f"br_{n}")
                nc.sync.dma_start(out=br[:], in_=ext[f"br_{n}"][:])
                e["W_sb"], e["a1r"], e["a2r"], e["br"] = W_sb, a1r, a2r, br

            def rowdot(dst_col, Wk, arep):
                """dst_col[128,1] f32 = sum_o Wk[:,o]*arep[:,o] (per part.)."""
                tmp = wp.tile([128, D], DT.float32, tag="rdtmp")
                nc.vector.tensor_tensor(out=tmp[:], in0=Wk, in1=arep,
                                        op=Alu.mult)
                nc.vector.tensor_reduce(out=dst_col, in_=tmp[:],
                                        axis=mybir.AxisListType.X, op=Alu.add)

            def hilo(dst_bf, src_f32, width):
                """Split f32 [128,w] into bf16 hi/lo pair stored at
                dst_bf [128, w, 2] (hi at [...,0], lo at [...,1])."""
                hi_f = wp.tile([128, width], DT.float32, tag="hilo_f")
                nc.vector.tensor_copy(out=dst_bf[:, :, 0], in_=src_f32)
                nc.vector.tensor_copy(out=hi_f[:], in_=dst_bf[:, :, 0])
                lo_f = wp.tile([128, width], DT.float32, tag="hilo_l")
                nc.vector.tensor_tensor(out=lo_f[:], in0=src_f32, in1=hi_f[:],
                                        op=Alu.subtract)
                nc.vector.tensor_copy(out=dst_bf[:, :, 1], in_=lo_f[:])

            for e in ET:
                n = e["name"]
                w1f = cp.tile([128, 2], DT.float32, tag=f"w1f_{n}")
                for k in (0, 1):
                    rowdot(w1f[:, k:k + 1], e["W_sb"][:, k, :], e["a1r"][:])
                w1b = cp.tile([128, 2, 2], DT.bfloat16, tag=f"w1b_{n}")
                hilo(w1b, w1f[:], 2)
                rhs = cp.tile([128, 2, D + 2], DT.bfloat16, tag=f"rhs_{n}")
                for k in (0, 1):
                    nc.vector.tensor_copy(out=rhs[:, k, 0:D],
                                          in_=e["W_sb"][:, k, :])
                    nc.vector.tensor_copy(out=rhs[:, k, D:D + 2],
                                          in_=w1b[:, k, :])
                e["rhs"] = rhs

            # w2 / bias for the s_dst grids (uses the OTHER etype's W and b)
            for e in ET:
                n = e["name"]
                o = ET[1 - e["i"]]
                w2f = cp.tile([128, 2], DT.float32, tag=f"w2f_{n}")
                for k in (0, 1):
                    rowdot(w2f[:, k:k + 1], o["W_sb"][:, k, :], e["a2r"][:])
                w2b = cp.tile([128, 2, 2], DT.bfloat16, tag=f"w2b_{n}")
                hilo(w2b, w2f[:], 2)
                e["w2b"] = w2b
                bias = cp.tile([128, 1], DT.float32, tag=f"bias_{n}")
                t2 = wp.tile([128, 1], DT.float32, tag="biast")
                rowdot(bias[:, 0:1], o["br"][:], e["a2r"][:])
                rowdot(t2[:, 0:1], e["br"][:], e["a1r"][:])
                nc.vector.tensor_tensor(out=bias[:], in0=bias[:], in1=t2[:],
                                        op=Alu.add)
                e["bias"] = bias

            # ---------------- GEMM (own slice) + AllGather -----------------
            for e in ET:
                if "gemm" not in PH:
                    break
                n = e["name"]
                for t in range(NSLICE):
                    lhsA = wp.tile([128, 128], DT.bfloat16, tag="lhsA")
                    nc.sync.dma_start(out=lhsA[:], in_=ext[f"xT_{n}"][t, 0:128, :])
                    lhsB = wp.tile([128, 128], DT.bfloat16, tag="lhsB")
                    nc.sync.dma_start(out=lhsB[:], in_=ext[f"xT_{n}"][t, 128:256, :])
                    ps = psA.tile([128, D + 2], DT.float32, tag="gemm")
                    nc.tensor.matmul(out=ps[:], lhsT=lhsA[:], rhs=e["rhs"][:, 0, :],
                                     start=True, stop=False)
                    nc.tensor.matmul(out=ps[:], lhsT=lhsB[:], rhs=e["rhs"][:, 1, :],
                                     start=False, stop=True)
                    tt = wp.tile([128, 260], DT.bfloat16, tag="ttile")
                    nc.scalar.activation(out=tt[:, 0:D], in_=ps[:, 0:D],
                                         func=ActF.Copy)
                    ttf = tt[:].bitcast(DT.float32)
                    nc.vector.tensor_reduce(out=ttf[:, SCOL_F32:SCOL_F32 + 1],
                                            in_=ps[:, D:D + 2],
                                            axis=mybir.AxisListType.X,
                                            op=Alu.add)
                    rows = min(128, SL_ROWS - t * 128)
                    nc.sync.dma_start(
                        out=e["loc"][t * 128:t * 128 + rows, 0:258],
                        in_=tt[0:rows, 0:258])
                if "ag" not in PH:
                    continue
                nc.gpsimd.collective_compute(
                    "AllGather", Alu.bypass,
                    replica_groups=[list(range(NCORES))],
                    ins=[e["loc"][:, :].opt()],
                    outs=[e["tbl"][:, :].opt()])

            # ---------------- s_dst grids (permuted GEMV) ------------------
            for e in ET:
                n = e["name"]
                BBtot = e["BB"][0] + e["BB"][1]
                grid = cp.tile([128, BBtot], DT.float32, tag=f"grid_{n}")
                if "grid" not in PH:
                    nc.vector.memset(grid[:], 0.0)
                    e["grid"] = grid
                    continue
                for g0 in range(0, BBtot, 8):
                    g1 = min(g0 + 8, BBtot)
                    ps = psB.tile([128, 2 * (g1 - g0)], DT.float32, tag="gemv")
                    for j, b in enumerate(range(g0, g1)):
                        xpA = wp.tile([128, 128], DT.bfloat16, tag="xpA")
                        nc.sync.dma_start(out=xpA[:], in_=ext[f"xperm_{n}"][b, 0:128, :])
                        xpB = wp.tile([128, 128], DT.bfloat16, tag="xpB")
                        nc.sync.dma_start(out=xpB[:], in_=ext[f"xperm_{n}"][b, 128:256, :])
                        nc.tensor.matmul(out=ps[:, 2 * j:2 * j + 2], lhsT=xpA[:],
                                         rhs=e["w2b"][:, 0, :], start=True, stop=False)
                        nc.tensor.matmul(out=ps[:, 2 * j:2 * j + 2], lhsT=xpB[:],
                                         rhs=e["w2b"][:, 1, :], start=False, stop=True)
                    nw = g1 - g0
                    nc.vector.tensor_reduce(
                        out=grid[:, g0:g1],
                        in_=ps[:, 0:2 * nw].rearrange("p (b two) -> p b two", two=2),
                        axis=mybir.AxisListType.X, op=Alu.add)
                    nc.vector.tensor_scalar(
                        out=grid[:, g0:g1], in0=grid[:, g0:g1],
                        scalar1=e["bias"][:, 0:1], scalar2=None, op0=Alu.add)
                e["grid"] = grid

            # ---------------------- edge phase -----------------------------
            for e in ET:
                if "edge" not in PH:
                    break
                n = e["name"]
                for h in (0, 1):
                    T, BBh = e["T"][h], e["BB"][h]
                    nth = e["nt"][h]
                    in_tbl = (e["tbl"][0:SPLIT, :] if h == 0
                              else e["tbl"][HI_BASE:N, :])
                    idx_sb = cp.tile([128, T * 8], DT.int16, tag=f"idx{h}_{n}")
                    nc.sync.dma_start(out=idx_sb[:], in_=ext[f"idx{h}_{n}"][:])
                    mask_sb = cp.tile([128, T], DT.float32, tag=f"mask{h}_{n}")
                    nc.sync.dma_start(out=mask_sb[:], in_=ext[f"mask{h}_{n}"][:])
                    E_sb = cp.tile([128, T], DT.float32, tag=f"E{h}_{n}")

                    # block schedule
                    starts = np.concatenate([[0], np.cumsum(nth)]).astype(int)
                    nblk_tiles = int(starts[-1])

                    nchunk = (T + CT - 1) // CT
                    gtiles = []
                    for ci in range(nchunk):
                        c0, c1 = ci * CT, min((ci + 1) * CT, T)
                        gt = gp.tile([128, c1 - c0, TCOLS], DT.bfloat16, tag="g")
                        for k in range(c0, c1, TPC):
                            nc.gpsimd.dma_gather(
                                out_ap=gt[:, k - c0:k - c0 + TPC, :],
                                in_ap=in_tbl,
                                idxs_ap=idx_sb[:, k * 8:(k + TPC) * 8],
                                num_idxs=TPC * 128, num_idxs_reg=TPC * 128,
                                elem_size=TCOLS)
                        gtiles.append((c0, c1, gt))
                        gf = gt[:].bitcast(DT.float32)
                        # logit pre: s_src + s_dst per block-piece
                        for b in range(BBh):
                            blo = max(int(starts[b]), c0)
                            bhi = min(int(starts[b + 1]), c1)
                            if blo >= bhi:
                                continue
                            gcol = b if h == 0 else e["BB"][0] + b
                            for lo in range(blo - (blo - c0) % TPC, bhi, TPC):
                                lo2 = max(lo, blo)
                                hi = min(lo + TPC - (lo - c0) % TPC, bhi)
                                if lo2 >= hi:
                                    continue
                                nc.vector.tensor_scalar(
                                    out=E_sb[:, lo2:hi],
                                    in0=gf[:, lo2 - c0:hi - c0, SCOL_F32],
                                    scalar1=e["grid"][:, gcol:gcol + 1],
                                    scalar2=None, op0=Alu.add)
                        ce = min(c1, nblk_tiles)
                        for k0 in range(c0, ce, TPC):
                            ke = min(k0 + TPC, ce)
                            lk = wp.tile([128, TPC], DT.float32, tag="lk")
                            w = ke - k0
                            nc.vector.tensor_scalar(
                                out=lk[:, 0:w], in0=E_sb[:, k0:ke],
                                scalar1=0.01, scalar2=None, op0=Alu.mult)
                            nc.vector.tensor_tensor(
                                out=lk[:, 0:w], in0=lk[:, 0:w],
                                in1=E_sb[:, k0:ke], op=Alu.max)
                            nc.scalar.activation(out=E_sb[:, k0:ke],
                                                 in_=lk[:, 0:w], func=ActF.Exp)
                            nc.vector.tensor_tensor(
                                out=E_sb[:, k0:ke], in0=E_sb[:, k0:ke],
                                in1=mask_sb[:, k0:ke], op=Alu.mult)

                    def gslice(t):
                        for (c0, c1, gt) in gtiles:
                            if c0 <= t < c1:
                                return gt[:, t - c0, 0:D]
                        raise AssertionError

                    for b in range(BBh):
                        t0, t1 = int(starts[b]), int(starts[b + 1])
                        ps = psE.tile([128, D], DT.float32, tag="edge")
                        for t in range(t0, t1):
                            if (t - t0) % 2 == 0:
                                dg = wp.tile([128, 128], DT.bfloat16, tag="diagV")
                                nc.vector.tensor_scalar(
                                    out=dg[:], in0=ident_sb[:],
                                    scalar1=E_sb[:, t:t + 1], scalar2=None,
                                    op0=Alu.mult)
                            else:
                                dg = wp.tile([128, 128], DT.bfloat16, tag="diagA")
                                nc.scalar.activation(
                                    out=dg[:], in_=ident_sb[:], func=ActF.Copy,
                                    scale=E_sb[:, t:t + 1])
                            nc.tensor.matmul(out=ps[:], lhsT=dg[:],
                                             rhs=gslice(t),
                                             start=(t == t0), stop=(t == t1 - 1))
                        hz = wp.tile([128, 257], DT.float32, tag="hz")
                        nc.scalar.activation(out=hz[:, 0:D], in_=ps[:],
                                             func=ActF.Copy)
                        # z = sum of E over the block's tiles (chunk pieces)
                        pieces = []
                        for (c0, c1, gt) in gtiles:
                            lo, hi = max(t0, c0), min(t1, c1)
                            if lo < hi:
                                pieces.append((lo, hi))
                        nc.vector.tensor_reduce(
                            out=hz[:, D:D + 1], in_=E_sb[:, pieces[0][0]:pieces[0][1]],
                            axis=mybir.AxisListType.X, op=Alu.add)
                        for (lo, hi) in pieces[1:]:
                            zt = wp.tile([128, 1], DT.float32, tag="zt")
                            nc.vector.tensor_reduce(
                                out=zt[:], in_=E_sb[:, lo:hi],
                                axis=mybir.AxisListType.X, op=Alu.add)
                            nc.vector.tensor_tensor(out=hz[:, D:D + 1],
                                                    in0=hz[:, D:D + 1],
                                                    in1=zt[:], op=Alu.add)
                        nc.sync.dma_start(out=ext[f"hz{h}_{n}"][b, :, :],
                                          in_=hz[:])
    nc.compile()
    return nc


# ----------------------------------------------------------------- kernel()
def _prepare(inputs, cfg):
    N = cfg["N"]
    ets = [
        dict(i=0, name="t", x_src="x_drug", x_dst="x_dis", W="W_t", b="b_t",
             a="a_t", src="src_t", dst="dst_t"),
        dict(i=1, name="rt", x_src="x_dis", x_dst="x_drug", W="W_rt", b="b_rt",
             a="a_rt", src="src_rt", dst="dst_rt"),
    ]
    sts = []
    for e in ets:
        st = _build_etype(np.asarray(inputs[e["src"]]),
                          np.asarray(inputs[e["dst"]]), cfg)
        st.update(e)
        sts.append(st)

    slice_rows = N // NCORES
    nslice = (slice_rows + 127) // 128
    gc = {"N": N, "SPLIT": cfg["SPLIT"], "HI_BASE": cfg["HI_BASE"],
          "slice_rows": slice_rows, "nslice_tiles": nslice,
          "etypes": [{"name": s["name"], "i": s["i"], "BB": s["BB"],
                      "nt": s["nt"], "T": s["T"]} for s in sts]}

    in_maps = []
    core_meta = []
    ident = np.eye(128, dtype=BF16)
    xbf = {k: np.asarray(inputs[k]).astype(BF16)
           for k in ("x_drug", "x_dis")}
    for c in range(NCORES):
        im = {"ident": ident}
        meta = {}
        for s in sts:
            n = s["name"]
            arr = _core_arrays(s, c, cfg)
            meta[n] = arr
            xs = xbf[s["x_src"]]
            im[f"xT_{n}"] = _tile_xT(xs[c * slice_rows:(c + 1) * slice_rows],
                                     nslice)
            im[f"W_{n}"] = np.asarray(inputs[s["W"]], np.float32)
            a = np.asarray(inputs[s["a"]], np.float32)
            im[f"a1r_{n}"] = np.tile(a[None, :D], (128, 1))
            im[f"a2r_{n}"] = np.tile(a[None, D:], (128, 1))
            im[f"br_{n}"] = np.tile(np.asarray(inputs[s["b"]],
                                               np.float32)[None, :], (128, 1))
            im[f"xperm_{n}"] = _perm_xT(xbf[s["x_dst"]],
                                        [arr["grid0"], arr["grid1"]])
            for h in (0, 1):
                im[f"idx{h}_{n}"] = _wrap_idx(arr[f"idx{h}"])
                im[f"mask{h}_{n}"] = np.ascontiguousarray(arr[f"mask{h}"])
        in_maps.append(im)
        core_meta.append(meta)
    return gc, sts, in_maps, core_meta


def _finalize(results, sts, core_meta, inputs, cfg):
    N = cfg["N"]
    outs = {}
    for s in sts:
        n = s["name"]
        h_un = np.zeros((N, D), np.float64)
        z = np.zeros(N, np.float64)
        for c in range(NCORES):
            for h in (0, 1):
                hz = np.asarray(results[c][f"hz{h}_{n}"], np.float64)
                hz2 = hz.reshape(-1, 257)
                grid = core_meta[c][n][f"grid{h}"].reshape(-1)
                m = grid >= 0
                rows = grid[m]
                h_un[rows] += hz2[m, :D]
                z[rows] += hz2[m, D]
        b = np.asarray(inputs[s["b"]], np.float64)
        has = z > 0
        h = np.zeros((N, D), np.float32)
        h[has] = (h_un[has] / z[has, None] + b[None, :]).astype(np.float32)
        outs[n] = h
    return outs


_CACHE = {}


def kernel(**inputs):
    n_nodes = int(np.asarray(inputs["x_drug"]).shape[0])
    cfg = _default_cfg(n_nodes)
    gc, sts, in_maps, core_meta = _prepare(inputs, cfg)

    key = os.environ.get("ATH_KERNEL_PHASES", "") + os.environ.get("ATH_GBUFS", "") + os.environ.get("ATH_PSE", "") + str(
        [(e["name"], e["BB"], [int(v) for v in e["nt"][0]],
          [int(v) for v in e["nt"][1]], e["T"]) for e in gc["etypes"]])
    if key not in _CACHE:
        _CACHE[key] = _build_nc(gc)
    nc = _CACHE[key]

    if os.environ.get("ATH_KERNEL_SIM"):
        from concourse.bass_interp import MultiCoreSim
        sim = MultiCoreSim(nc, num_cores=NCORES, require_finite=False,
                           require_nnan=False)
        for c in range(NCORES):
            for k, v in in_maps[c].items():
                sim.cores[c].tensor(k)[:] = v
        sim.simulate()
        results = []
        for c in range(NCORES):
            out = {}
            for s in sts:
                for h in (0, 1):
                    name = f"hz{h}_{s['name']}"
                    out[name] = np.array(sim.cores[c].mem_tensor(name))
            results.append(out)
    else:
        from concourse.bass_utils import run_bass_kernel_spmd
        res = run_bass_kernel_spmd(nc, in_maps, core_ids=list(range(NCORES)))
        results = res.results

    outs = _finalize(results, sts, core_meta, inputs, cfg)
    return outs["rt"], outs["t"]     # (h_drug, h_dis)


# revision 11
# speedup vs baseline: 1.2264x; 1.2264x over previous
"""Trainium2 Bass kernel for nn_AttHeteroRGCNLayer (GAT-style hetero GNN layer).

Strategy (8 NeuronCores, SPMD):
  - dst-sharded edge phase: dsts are degree-snake-dealt to cores; every edge of
    a dst lives on one core, so segment softmax is core-local (no collectives
    for softmax statistics).
  - per core, edges are split by src range (int16 gather-index limit) into two
    independent structures; each packs dsts by degree into 128-slot blocks.
    A (block, slot, tile) grid assigns edge t of dst-slot p to lane p of tile
    t; the scatter-sum becomes diag(E) matmuls accumulating in PSUM.
  - projections are data-parallel GEMMs + AllGather of a bf16 node table
    [N x 384]: cols 0..255 = bf16(x @ W), f32 (x @ (W a1)) punned at 256-7.
  - per-slot s_dst comes from a small GEMV over host-permuted x columns.
  - host merges the two halves' unnormalized (h|z) grids, divides, adds bias.
"""

import os
import sys
import numpy as np

for _p in ("/opt/trn_rl_repo", "/root/.axon_site/_ro/trn_rl_repo"):
    if os.path.isdir(_p) and _p not in sys.path:
        sys.path.append(_p)

import ml_dtypes  # noqa: E402

BF16 = ml_dtypes.bfloat16
D = 256
NCORES = 8
TCOLS = 384            # table row = 768B (256 msg bf16 | s_src f32 | junk)
SCOL_F32 = 128         # f32 column of s_src in the 192-col f32 view of a row
TPC = 8                # tiles per dma_gather call (1024 idx: SWDGE ring limit)
CT = 40                # tiles per SBUF gather chunk (must be multiple of TPC)


def _default_cfg(n_nodes):
    split = 32768 if n_nodes > 32768 else n_nodes
    return {
        "N": n_nodes,
        "SPLIT": split,                      # half0: src < SPLIT
        "HI_BASE": max(0, n_nodes - 32768),  # half1 idx = src - HI_BASE
    }


# ----------------------------------------------------------------- host prep
def _build_etype(src, dst, cfg):
    N = cfg["N"]
    SPLIT = cfg["SPLIT"]
    deg = np.bincount(dst, minlength=N)
    order = np.argsort(-deg, kind="stable")
    core_of = np.empty(N, np.int32)
    fwd = np.arange(NCORES)
    rev = fwd[::-1]
    for i in range(0, N, 2 * NCORES):
        blk = order[i:i + NCORES]
        core_of[blk] = fwd[:len(blk)]
        blk = order[i + NCORES:i + 2 * NCORES]
        core_of[blk] = rev[:len(blk)]

    ecore = core_of[dst]
    half = (src >= SPLIT).astype(np.int8)

    percore = []
    BB = [1, 1]
    for c in range(NCORES):
        cdsts = np.where(core_of == c)[0]
        ent = {}
        for h in (0, 1):
            m = (ecore == c) & (half == h)
            hsrc, hdst = src[m], dst[m]
            hdeg = np.bincount(hdst, minlength=N)[cdsts]
            oo = np.argsort(-hdeg, kind="stable")
            ent[f"slots{h}"] = cdsts[oo]
            ent[f"sdeg{h}"] = hdeg[oo]
            ent[f"src{h}"] = hsrc
            ent[f"dst{h}"] = hdst
            nb = max(1, int(np.ceil(max(1, int((hdeg > 0).sum())) / 128)))
            BB[h] = max(BB[h], nb)
        percore.append(ent)

    nt = [np.ones(BB[0], np.int64), np.ones(BB[1], np.int64)]
    for c in range(NCORES):
        for h in (0, 1):
            sdeg = percore[c][f"sdeg{h}"]
            for b in range(BB[h]):
                blk = sdeg[b * 128:(b + 1) * 128]
                if len(blk) and blk.max() > nt[h][b]:
                    nt[h][b] = int(blk.max())
    T = [int(np.ceil(int(nt[h].sum()) / TPC) * TPC) for h in (0, 1)]
    return {"BB": BB, "nt": nt, "T": T, "percore": percore}


def _core_arrays(st, c, cfg):
    HI_BASE = cfg["HI_BASE"]
    per = st["percore"][c]
    res = {}
    for h in (0, 1):
        BBh, nth, T = st["BB"][h], st["nt"][h], st["T"][h]
        slots, sdeg = per[f"slots{h}"], per[f"sdeg{h}"]
        hsrc, hdst = per[f"src{h}"], per[f"dst{h}"]
        o = np.argsort(hdst, kind="stable")
        s_sorted, d_sorted = hsrc[o], hdst[o]
        uq, first = np.unique(d_sorted, return_index=True)
        start_of = dict(zip(uq.tolist(), first.tolist()))

        idx = np.zeros((T, 128), np.int32)
        mask = np.zeros((128, T), np.float32)       # (lane, tile)
        grid_nodes = np.full((BBh, 128), -1, np.int64)
        t0 = 0
        for b in range(BBh):
            bs = slots[b * 128:(b + 1) * 128]
            bd = sdeg[b * 128:(b + 1) * 128]
            ntb = int(nth[b])
            for p in range(len(bs)):
                dnode = int(bs[p])
                dg = int(bd[p])
                grid_nodes[b, p] = dnode
                if dg:
                    f = start_of[dnode]
                    v = s_sorted[f:f + dg]
                    idx[t0:t0 + dg, p] = v if h == 0 else v - HI_BASE
                    mask[p, t0:t0 + dg] = 1.0
            t0 += ntb
        res[f"idx{h}"] = idx
        res[f"mask{h}"] = mask
        res[f"grid{h}"] = grid_nodes
    return res


def _wrap_idx(idx_tl):
    """(tile, lane) int32 -> dma_gather wrapped int16 [128, n/16] (x8 groups)."""
    flat = idx_tl.reshape(-1)
    n = len(flat)
    a = np.zeros((16, n // 16), np.int16)
    a[np.arange(n) % 16, np.arange(n) // 16] = flat.astype(np.int16)
    return np.ascontiguousarray(np.tile(a, (8, 1)))


def _tile_xT(x_bf, ntiles):
    """x [M, D] bf16 -> tile-major [ntiles, D, 128] (zero padded)."""
    out = np.zeros((ntiles, D, 128), BF16)
    for t in range(ntiles):
        rows = x_bf[t * 128:(t + 1) * 128]
        if rows.shape[0]:
            out[t, :, :rows.shape[0]] = rows.T
    return out


def _perm_xT(x_bf, grids):
    """Permuted x columns for the s_dst GEMV: [BBtot, D, 128]."""
    mats = []
    for grid in grids:
        for b in range(grid.shape[0]):
            m = np.zeros((D, 128), BF16)
            gd = grid[b]
            valid = gd >= 0
            if valid.any():
                m[:, valid] = x_bf[gd[valid]].T
            mats.append(m)
    return np.stack(mats)


# ------------------------------------------------------------- device build
def _build_nc(gc):
    import concourse.bass as bass  # noqa: F401
    import concourse.mybir as mybir
    from concourse import bacc
    from concourse.tile import TileContext

    DT = mybir.dt
    Alu = mybir.AluOpType
    ActF = mybir.ActivationFunctionType
    N = gc["N"]
    SPLIT = gc["SPLIT"]
    HI_BASE = gc["HI_BASE"]
    SL_ROWS = gc["slice_rows"]
    NSLICE = gc["nslice_tiles"]
    ET = gc["etypes"]

    PH = set(os.environ.get("ATH_KERNEL_PHASES", "gemm,ag,grid,edge").split(","))
    nc = bacc.Bacc("TRN2", target_bir_lowering=False, debug=False,
                   num_devices=NCORES)
    ext = {}

    def din(name, shape, dt):
        ext[name] = nc.dram_tensor(name, shape, dt, kind="ExternalInput").ap()
        return ext[name]

    def dout(name, shape, dt):
        ext[name] = nc.dram_tensor(name, shape, dt, kind="ExternalOutput").ap()
        return ext[name]

    ident_e = din("ident", [128, 128], DT.bfloat16)
    for e in ET:
        n = e["name"]
        din(f"xT_{n}", [NSLICE, D, 128], DT.bfloat16)
        din(f"W_{n}", [D, D], DT.float32)
        din(f"a1r_{n}", [128, D], DT.float32)
        din(f"a2r_{n}", [128, D], DT.float32)
        din(f"br_{n}", [128, D], DT.float32)
        BBtot = e["BB"][0] + e["BB"][1]
        din(f"xperm_{n}", [BBtot, D, 128], DT.bfloat16)
        for h in (0, 1):
            din(f"idx{h}_{n}", [128, e["T"][h] * 8], DT.int16)
            din(f"mask{h}_{n}", [128, e["T"][h]], DT.float32)
            dout(f"hz{h}_{n}", [e["BB"][h], 128, 257], DT.bfloat16)
        e["tbl"] = nc.dram_tensor(f"tbl_{n}", [N, TCOLS], DT.bfloat16,
                                  addr_space="Shared")
        e["loc"] = nc.dram_tensor(f"loc_{n}", [SL_ROWS, TCOLS], DT.bfloat16)

    with TileContext(nc) as tc:
        with tc.tile_pool(name="const", bufs=1) as cp, \
             tc.tile_pool(name="work", bufs=2) as wp, \
             tc.tile_pool(name="gbuf", bufs=int(os.environ.get("ATH_GBUFS", "3"))) as gp, \
             tc.tile_pool(name="psA", bufs=2, space="PSUM") as psA, \
             tc.tile_pool(name="psB", bufs=2, space="PSUM") as psB, \
             tc.tile_pool(name="psE", bufs=int(os.environ.get("ATH_PSE", "4")), space="PSUM") as psE:

            ident_sb = cp.tile([128, 128], DT.bfloat16, tag="ident")
            nc.sync.dma_start(out=ident_sb[:], in_=ident_e[:])

            # -------- per-etype weight prep (W, w1 hi/lo, rhs tiles) -------
            for e in ET:
                n = e["name"]
                W_sb = cp.tile([128, 2, D], DT.float32, tag=f"W_{n}")
                nc.sync.dma_start(out=W_sb[:, 0, :], in_=ext[f"W_{n}"][0:128, :])
                nc.sync.dma_start(out=W_sb[:, 1, :], in_=ext[f"W_{n}"][128:256, :])
                a1r = cp.tile([128, D], DT.float32, tag=f"a1r_{n}")
                nc.sync.dma_start(out=a1r[:], in_=ext[f"a1r_{n}"][:])
                a2r = cp.tile([128, D], DT.float32, tag=f"a2r_{n}")
                nc.sync.dma_start(out=a2r[:], in_=ext[f"a2r_{n}"][:])
                br = cp.tile([128, D], DT.float32, tag=f"br_{n}")
                nc.sync.dma_start(out=br[:], in_=ext[f"br_{n}"][:])
                e["W_sb"], e["a1r"], e["a2r"], e["br"] = W_sb, a1r, a2r, br

            def rowdot(dst_col, Wk, arep):
                """dst_col[128,1] f32 = sum_o Wk[:,o]*arep[:,o] (per part.)."""
                tmp = wp.tile([128, D], DT.float32, tag="rdtmp")
                nc.vector.tensor_tensor(out=tmp[:], in0=Wk, in1=arep,
                                        op=Alu.mult)
                nc.vector.tensor_reduce(out=dst_col, in_=tmp[:],
                                        axis=mybir.AxisListType.X, op=Alu.add)

            def hilo(dst_bf, src_f32, width):
                """Split f32 [128,w] into bf16 hi/lo pair stored at
                dst_bf [128, w, 2] (hi at [...,0], lo at [...,1])."""
                hi_f = wp.tile([128, width], DT.float32, tag="hilo_f")
                nc.vector.tensor_copy(out=dst_bf[:, :, 0], in_=src_f32)
                nc.vector.tensor_copy(out=hi_f[:], in_=dst_bf[:, :, 0])
                lo_f = wp.tile([128, width], DT.float32, tag="hilo_l")
                nc.vector.tensor_tensor(out=lo_f[:], in0=src_f32, in1=hi_f[:],
                                        op=Alu.subtract)
                nc.vector.tensor_copy(out=dst_bf[:, :, 1], in_=lo_f[:])

            for e in ET:
                n = e["name"]
                w1f = cp.tile([128, 2], DT.float32, tag=f"w1f_{n}")
                for k in (0, 1):
                    rowdot(w1f[:, k:k + 1], e["W_sb"][:, k, :], e["a1r"][:])
                w1b = cp.tile([128, 2, 2], DT.bfloat16, tag=f"w1b_{n}")
                hilo(w1b, w1f[:], 2)
                rhs = cp.tile([128, 2, D + 2], DT.bfloat16, tag=f"rhs_{n}")
                for k in (0, 1):
                    nc.vector.tensor_copy(out=rhs[:, k, 0:D],
                                          in_=e["W_sb"][:, k, :])
                    nc.vector.tensor_copy(out=rhs[:, k, D:D + 2],
                                          in_=w1b[:, k, :])
                e["rhs"] = rhs

            # w2 / bias for the s_dst grids (uses the OTHER etype's W and b)
            for e in ET:
                n = e["name"]
                o = ET[1 - e["i"]]
                w2f = cp.tile([128, 2], DT.float32, tag=f"w2f_{n}")
                for k in (0, 1):
                    rowdot(w2f[:, k:k + 1], o["W_sb"][:, k, :], e["a2r"][:])
                w2b = cp.tile([128, 2, 2], DT.bfloat16, tag=f"w2b_{n}")
                hilo(w2b, w2f[:], 2)
                e["w2b"] = w2b
                bias = cp.tile([128, 1], DT.float32, tag=f"bias_{n}")
                t2 = wp.tile([128, 1], DT.float32, tag="biast")
                rowdot(bias[:, 0:1], o["br"][:], e["a2r"][:])
                rowdot(t2[:, 0:1], e["br"][:], e["a1r"][:])
                nc.vector.tensor_tensor(out=bias[:], in0=bias[:], in1=t2[:],
                                        op=Alu.add)
                e["bias"] = bias

            # ---------------- GEMM (own slice) + AllGather -----------------
            for e in ET:
                if "gemm" not in PH:
                    break
                n = e["name"]
                for t in range(NSLICE):
                    lhsA = wp.tile([128, 128], DT.bfloat16, tag="lhsA")
                    nc.sync.dma_start(out=lhsA[:], in_=ext[f"xT_{n}"][t, 0:128, :])
                    lhsB = wp.tile([128, 128], DT.bfloat16, tag="lhsB")
                    nc.sync.dma_start(out=lhsB[:], in_=ext[f"xT_{n}"][t, 128:256, :])
                    ps = psA.tile([128, D + 2], DT.float32, tag="gemm")
                    nc.tensor.matmul(out=ps[:], lhsT=lhsA[:], rhs=e["rhs"][:, 0, :],
                                     start=True, stop=False)
                    nc.tensor.matmul(out=ps[:], lhsT=lhsB[:], rhs=e["rhs"][:, 1, :],
                                     start=False, stop=True)
                    tt = wp.tile([128, 260], DT.bfloat16, tag="ttile")
                    nc.scalar.activation(out=tt[:, 0:D], in_=ps[:, 0:D],
                                         func=ActF.Copy)
                    ttf = tt[:].bitcast(DT.float32)
                    nc.vector.tensor_reduce(out=ttf[:, SCOL_F32:SCOL_F32 + 1],
                                            in_=ps[:, D:D + 2],
                                            axis=mybir.AxisListType.X,
                                            op=Alu.add)
                    rows = min(128, SL_ROWS - t * 128)
                    nc.sync.dma_start(
                        out=e["loc"][t * 128:t * 128 + rows, 0:258],
                        in_=tt[0:rows, 0:258])
                if "ag" not in PH:
                    continue
                nc.gpsimd.collective_compute(
                    "AllGather", Alu.bypass,
                    replica_groups=[list(range(NCORES))],
                    ins=[e["loc"][:, :].opt()],
                    outs=[e["tbl"][:, :].opt()])

            # ---------------- s_dst grids (permuted GEMV) ------------------
            for e in ET:
                n = e["name"]
                BBtot = e["BB"][0] + e["BB"][1]
                grid = cp.tile([128, BBtot], DT.float32, tag=f"grid_{n}")
                if "grid" not in PH:
                    nc.vector.memset(grid[:], 0.0)
                    e["grid"] = grid
                    continue
                for g0 in range(0, BBtot, 8):
                    g1 = min(g0 + 8, BBtot)
                    ps = psB.tile([128, 2 * (g1 - g0)], DT.float32, tag="gemv")
                    for j, b in enumerate(range(g0, g1)):
                        xpA = wp.tile([128, 128], DT.bfloat16, tag="xpA")
                        nc.sync.dma_start(out=xpA[:], in_=ext[f"xperm_{n}"][b, 0:128, :])
                        xpB = wp.tile([128, 128], DT.bfloat16, tag="xpB")
                        nc.sync.dma_start(out=xpB[:], in_=ext[f"xperm_{n}"][b, 128:256, :])
                        nc.tensor.matmul(out=ps[:, 2 * j:2 * j + 2], lhsT=xpA[:],
                                         rhs=e["w2b"][:, 0, :], start=True, stop=False)
                        nc.tensor.matmul(out=ps[:, 2 * j:2 * j + 2], lhsT=xpB[:],
                                         rhs=e["w2b"][:, 1, :], start=False, stop=True)
                    nw = g1 - g0
                    nc.vector.tensor_reduce(
                        out=grid[:, g0:g1],
                        in_=ps[:, 0:2 * nw].rearrange("p (b two) -> p b two", two=2),
                        axis=mybir.AxisListType.X, op=Alu.add)
                    nc.vector.tensor_scalar(
                        out=grid[:, g0:g1], in0=grid[:, g0:g1],
                        scalar1=e["bias"][:, 0:1], scalar2=None, op0=Alu.add)
                e["grid"] = grid

            # ---------------------- edge phase -----------------------------
            for e in ET:
                if "edge" not in PH:
                    break
                n = e["name"]
                for h in (0, 1):
                    T, BBh = e["T"][h], e["BB"][h]
                    nth = e["nt"][h]
                    in_tbl = (e["tbl"][0:SPLIT, :] if h == 0
                              else e["tbl"][HI_BASE:N, :])
                    idx_sb = cp.tile([128, T * 8], DT.int16, tag=f"idx{h}_{n}")
                    nc.sync.dma_start(out=idx_sb[:], in_=ext[f"idx{h}_{n}"][:])
                    mask_sb = cp.tile([128, T], DT.float32, tag=f"mask{h}_{n}")
                    nc.sync.dma_start(out=mask_sb[:], in_=ext[f"mask{h}_{n}"][:])
                    E_sb = cp.tile([128, T], DT.float32, tag=f"E{h}_{n}")

                    # block schedule
                    starts = np.concatenate([[0], np.cumsum(nth)]).astype(int)
                    nblk_tiles = int(starts[-1])

                    nchunk = (T + CT - 1) // CT
                    gtiles = []
                    for ci in range(nchunk):
                        c0, c1 = ci * CT, min((ci + 1) * CT, T)
                        gt = gp.tile([128, c1 - c0, TCOLS], DT.bfloat16, tag="g")
                        for k in range(c0, c1, TPC):
                            nc.gpsimd.dma_gather(
                                out_ap=gt[:, k - c0:k - c0 + TPC, :],
                                in_ap=in_tbl,
                                idxs_ap=idx_sb[:, k * 8:(k + TPC) * 8],
                                num_idxs=TPC * 128, num_idxs_reg=TPC * 128,
                                elem_size=TCOLS)
                        gtiles.append((c0, c1, gt))
                        gf = gt[:].bitcast(DT.float32)
                        # logit pre: s_src + s_dst per block-piece
                        for b in range(BBh):
                            blo = max(int(starts[b]), c0)
                            bhi = min(int(starts[b + 1]), c1)
                            if blo >= bhi:
                                continue
                            gcol = b if h == 0 else e["BB"][0] + b
                            for lo in range(blo - (blo - c0) % TPC, bhi, TPC):
                                lo2 = max(lo, blo)
                                hi = min(lo + TPC - (lo - c0) % TPC, bhi)
                                if lo2 >= hi:
                                    continue
                                nc.vector.tensor_scalar(
                                    out=E_sb[:, lo2:hi],
                                    in0=gf[:, lo2 - c0:hi - c0, SCOL_F32],
                                    scalar1=e["grid"][:, gcol:gcol + 1],
                                    scalar2=None, op0=Alu.add)
                        ce = min(c1, nblk_tiles)
                        for k0 in range(c0, ce, TPC):
                            ke = min(k0 + TPC, ce)
                            lk = wp.tile([128, TPC], DT.float32, tag="lk")
                            w = ke - k0
                            nc.vector.tensor_scalar(
                                out=lk[:, 0:w], in0=E_sb[:, k0:ke],
                                scalar1=0.01, scalar2=None, op0=Alu.mult)
                            nc.vector.tensor_tensor(
                                out=lk[:, 0:w], in0=lk[:, 0:w],
                                in1=E_sb[:, k0:ke], op=Alu.max)
                            nc.scalar.activation(out=E_sb[:, k0:ke],
                                                 in_=lk[:, 0:w], func=ActF.Exp)
                            nc.vector.tensor_tensor(
                                out=E_sb[:, k0:ke], in0=E_sb[:, k0:ke],
                                in1=mask_sb[:, k0:ke], op=Alu.mult)

                    def gslice(t):
                        for (c0, c1, gt) in gtiles:
                            if c0 <= t < c1:
                                return gt[:, t - c0, 0:D]
                        raise AssertionError

                    for b in range(BBh):
                        t0, t1 = int(starts[b]), int(starts[b + 1])
                        ps = psE.tile([128, D], DT.float32, tag="edge")
                        for t in range(t0, t1):
                            if (t - t0) % 2 == 0:
                                dg = wp.tile([128, 128], DT.bfloat16, tag="diagV")
                                nc.vector.tensor_scalar(
                                    out=dg[:], in0=ident_sb[:],
                                    scalar1=E_sb[:, t:t + 1], scalar2=None,
                                    op0=Alu.mult)
                            else:
                                dg = wp.tile([128, 128], DT.bfloat16, tag="diagA")
                                nc.scalar.activation(
                                    out=dg[:], in_=ident_sb[:], func=ActF.Copy,
                                    scale=E_sb[:, t:t + 1])
                            nc.tensor.matmul(out=ps[:], lhsT=dg[:],
                                             rhs=gslice(t),
                                             start=(t == t0), stop=(t == t1 - 1))
                        hz = wp.tile([128, 257], DT.bfloat16, tag="hz")
                        nc.scalar.activation(out=hz[:, 0:D], in_=ps[:],
                                             func=ActF.Copy)
                        # z = sum of E over the block's tiles (chunk pieces)
                        pieces = []
                        for (c0, c1, gt) in gtiles:
                            lo, hi = max(t0, c0), min(t1, c1)
                            if lo < hi:
                                pieces.append((lo, hi))
                        zf = wp.tile([128, 1], DT.float32, tag="zf")
                        nc.vector.tensor_reduce(
                            out=zf[:], in_=E_sb[:, pieces[0][0]:pieces[0][1]],
                            axis=mybir.AxisListType.X, op=Alu.add)
                        for (lo, hi) in pieces[1:]:
                            zt = wp.tile([128, 1], DT.float32, tag="zt")
                            nc.vector.tensor_reduce(
                                out=zt[:], in_=E_sb[:, lo:hi],
                                axis=mybir.AxisListType.X, op=Alu.add)
                            nc.vector.tensor_tensor(out=zf[:], in0=zf[:],
                                                    in1=zt[:], op=Alu.add)
                        nc.vector.tensor_copy(out=hz[:, D:D + 1], in_=zf[:])
                        nc.sync.dma_start(out=ext[f"hz{h}_{n}"][b, :, :],
                                          in_=hz[:])
    nc.compile()
    return nc


# ----------------------------------------------------------------- kernel()
def _prepare(inputs, cfg):
    N = cfg["N"]
    ets = [
        dict(i=0, name="t", x_src="x_drug", x_dst="x_dis", W="W_t", b="b_t",
             a="a_t", src="src_t", dst="dst_t"),
        dict(i=1, name="rt", x_src="x_dis", x_dst="x_drug", W="W_rt", b="b_rt",
             a="a_rt", src="src_rt", dst="dst_rt"),
    ]
    sts = []
    for e in ets:
        st = _build_etype(np.asarray(inputs[e["src"]]),
                          np.asarray(inputs[e["dst"]]), cfg)
        st.update(e)
        sts.append(st)

    slice_rows = N // NCORES
    nslice = (slice_rows + 127) // 128
    gc = {"N": N, "SPLIT": cfg["SPLIT"], "HI_BASE": cfg["HI_BASE"],
          "slice_rows": slice_rows, "nslice_tiles": nslice,
          "etypes": [{"name": s["name"], "i": s["i"], "BB": s["BB"],
                      "nt": s["nt"], "T": s["T"]} for s in sts]}

    in_maps = []
    core_meta = []
    ident = np.eye(128, dtype=BF16)
    xbf = {k: np.asarray(inputs[k]).astype(BF16)
           for k in ("x_drug", "x_dis")}
    for c in range(NCORES):
        im = {"ident": ident}
        meta = {}
        for s in sts:
            n = s["name"]
            arr = _core_arrays(s, c, cfg)
            meta[n] = arr
            xs = xbf[s["x_src"]]
            im[f"xT_{n}"] = _tile_xT(xs[c * slice_rows:(c + 1) * slice_rows],
                                     nslice)
            im[f"W_{n}"] = np.asarray(inputs[s["W"]], np.float32)
            a = np.asarray(inputs[s["a"]], np.float32)
            im[f"a1r_{n}"] = np.tile(a[None, :D], (128, 1))
            im[f"a2r_{n}"] = np.tile(a[None, D:], (128, 1))
            im[f"br_{n}"] = np.tile(np.asarray(inputs[s["b"]],
                                               np.float32)[None, :], (128, 1))
            im[f"xperm_{n}"] = _perm_xT(xbf[s["x_dst"]],
                                        [arr["grid0"], arr["grid1"]])
            for h in (0, 1):
                im[f"idx{h}_{n}"] = _wrap_idx(arr[f"idx{h}"])
                im[f"mask{h}_{n}"] = np.ascontiguousarray(arr[f"mask{h}"])
        in_maps.append(im)
        core_meta.append(meta)
    return gc, sts, in_maps, core_meta


def _finalize(results, sts, core_meta, inputs, cfg):
    N = cfg["N"]
    outs = {}
    for s in sts:
        n = s["name"]
        h_un = np.zeros((N, D), np.float64)
        z = np.zeros(N, np.float64)
        for c in range(NCORES):
            for h in (0, 1):
                hz = np.asarray(results[c][f"hz{h}_{n}"], np.float64)
                hz2 = hz.reshape(-1, 257)
                grid = core_meta[c][n][f"grid{h}"].reshape(-1)
                m = grid >= 0
                rows = grid[m]
                h_un[rows] += hz2[m, :D]
                z[rows] += hz2[m, D]
        b = np.asarray(inputs[s["b"]], np.float64)
        has = z > 0
        h = np.zeros((N, D), np.float32)
        h[has] = (h_un[has] / z[has, None] + b[None, :]).astype(np.float32)
        outs[n] = h
    return outs


_CACHE = {}


def kernel(**inputs):
    n_nodes = int(np.asarray(inputs["x_drug"]).shape[0])
    cfg = _default_cfg(n_nodes)
    gc, sts, in_maps, core_meta = _prepare(inputs, cfg)

    key = os.environ.get("ATH_KERNEL_PHASES", "") + os.environ.get("ATH_GBUFS", "") + os.environ.get("ATH_PSE", "") + str(
        [(e["name"], e["BB"], [int(v) for v in e["nt"][0]],
          [int(v) for v in e["nt"][1]], e["T"]) for e in gc["etypes"]])
    if key not in _CACHE:
        _CACHE[key] = _build_nc(gc)
    nc = _CACHE[key]

    if os.environ.get("ATH_KERNEL_SIM"):
        from concourse.bass_interp import MultiCoreSim
        sim = MultiCoreSim(nc, num_cores=NCORES, require_finite=False,
                           require_nnan=False)
        for c in range(NCORES):
            for k, v in in_maps[c].items():
                sim.cores[c].tensor(k)[:] = v
        sim.simulate()
        results = []
        for c in range(NCORES):
            out = {}
            for s in sts:
                for h in (0, 1):
                    name = f"hz{h}_{s['name']}"
                    out[name] = np.array(sim.cores[c].mem_tensor(name))
            results.append(out)
    else:
        from concourse.bass_utils import run_bass_kernel_spmd
        res = run_bass_kernel_spmd(nc, in_maps, core_ids=list(range(NCORES)))
        results = res.results

    outs = _finalize(results, sts, core_meta, inputs, cfg)
    return outs["rt"], outs["t"]     # (h_drug, h_dis)


# revision 12
# speedup vs baseline: 1.5367x; 1.2531x over previous
"""Trainium2 Bass kernel for nn_AttHeteroRGCNLayer (GAT-style hetero GNN layer).

Strategy (8 NeuronCores, SPMD):
  - dst-sharded edge phase: dsts are degree-snake-dealt to cores; every edge of
    a dst lives on one core, so segment softmax is core-local (no collectives
    for softmax statistics).
  - per core, edges are split by src range (int16 gather-index limit) into two
    independent structures; each packs dsts by degree into 128-slot blocks.
    A (block, slot, tile) grid assigns edge t of dst-slot p to lane p of tile
    t; the scatter-sum becomes diag(E) matmuls accumulating in PSUM.
  - projections are data-parallel GEMMs + AllGather of a bf16 node table
    [N x 384]: cols 0..255 = bf16(x @ W), f32 (x @ (W a1)) punned at 256-7.
  - per-slot s_dst comes from a small GEMV over host-permuted x columns.
  - host merges the two halves' unnormalized (h|z) grids, divides, adds bias.
"""

import os
import sys
import numpy as np

for _p in ("/opt/trn_rl_repo", "/root/.axon_site/_ro/trn_rl_repo"):
    if os.path.isdir(_p) and _p not in sys.path:
        sys.path.append(_p)

import ml_dtypes  # noqa: E402

BF16 = ml_dtypes.bfloat16
D = 256
NCORES = 8
TCOLS = 384            # table row = 768B (256 msg bf16 | s_src f32 | junk)
SCOL_F32 = 128         # f32 column of s_src in the 192-col f32 view of a row
TPC = 8                # tiles per dma_gather call (1024 idx: SWDGE ring limit)
CT = 24                # tiles per SBUF gather chunk (must be multiple of TPC)


def _default_cfg(n_nodes):
    split = 32768 if n_nodes > 32768 else n_nodes
    return {
        "N": n_nodes,
        "SPLIT": split,                      # half0: src < SPLIT
        "HI_BASE": max(0, n_nodes - 32768),  # half1 idx = src - HI_BASE
    }


# ----------------------------------------------------------------- host prep
def _build_etype(src, dst, cfg):
    N = cfg["N"]
    SPLIT = cfg["SPLIT"]
    deg = np.bincount(dst, minlength=N)
    order = np.argsort(-deg, kind="stable")
    core_of = np.empty(N, np.int32)
    fwd = np.arange(NCORES)
    rev = fwd[::-1]
    for i in range(0, N, 2 * NCORES):
        blk = order[i:i + NCORES]
        core_of[blk] = fwd[:len(blk)]
        blk = order[i + NCORES:i + 2 * NCORES]
        core_of[blk] = rev[:len(blk)]

    ecore = core_of[dst]
    half = (src >= SPLIT).astype(np.int8)

    percore = []
    BB = [1, 1]
    for c in range(NCORES):
        cdsts = np.where(core_of == c)[0]
        ent = {}
        for h in (0, 1):
            m = (ecore == c) & (half == h)
            hsrc, hdst = src[m], dst[m]
            hdeg = np.bincount(hdst, minlength=N)[cdsts]
            oo = np.argsort(-hdeg, kind="stable")
            ent[f"slots{h}"] = cdsts[oo]
            ent[f"sdeg{h}"] = hdeg[oo]
            ent[f"src{h}"] = hsrc
            ent[f"dst{h}"] = hdst
            nb = max(1, int(np.ceil(max(1, int((hdeg > 0).sum())) / 128)))
            BB[h] = max(BB[h], nb)
        percore.append(ent)

    nt = [np.ones(BB[0], np.int64), np.ones(BB[1], np.int64)]
    for c in range(NCORES):
        for h in (0, 1):
            sdeg = percore[c][f"sdeg{h}"]
            for b in range(BB[h]):
                blk = sdeg[b * 128:(b + 1) * 128]
                if len(blk) and blk.max() > nt[h][b]:
                    nt[h][b] = int(blk.max())
    T = [int(np.ceil(int(nt[h].sum()) / TPC) * TPC) for h in (0, 1)]
    return {"BB": BB, "nt": nt, "T": T, "percore": percore}


def _core_arrays(st, c, cfg):
    HI_BASE = cfg["HI_BASE"]
    per = st["percore"][c]
    res = {}
    for h in (0, 1):
        BBh, nth, T = st["BB"][h], st["nt"][h], st["T"][h]
        slots, sdeg = per[f"slots{h}"], per[f"sdeg{h}"]
        hsrc, hdst = per[f"src{h}"], per[f"dst{h}"]
        o = np.argsort(hdst, kind="stable")
        s_sorted, d_sorted = hsrc[o], hdst[o]
        uq, first = np.unique(d_sorted, return_index=True)
        start_of = dict(zip(uq.tolist(), first.tolist()))

        idx = np.zeros((T, 128), np.int32)
        mask = np.zeros((128, T), np.float32)       # (lane, tile)
        grid_nodes = np.full((BBh, 128), -1, np.int64)
        t0 = 0
        for b in range(BBh):
            bs = slots[b * 128:(b + 1) * 128]
            bd = sdeg[b * 128:(b + 1) * 128]
            ntb = int(nth[b])
            for p in range(len(bs)):
                dnode = int(bs[p])
                dg = int(bd[p])
                grid_nodes[b, p] = dnode
                if dg:
                    f = start_of[dnode]
                    v = s_sorted[f:f + dg]
                    idx[t0:t0 + dg, p] = v if h == 0 else v - HI_BASE
                    mask[p, t0:t0 + dg] = 1.0
            t0 += ntb
        res[f"idx{h}"] = idx
        res[f"mask{h}"] = mask
        res[f"grid{h}"] = grid_nodes
    return res


def _wrap_idx(idx_tl):
    """(tile, lane) int32 -> dma_gather wrapped int16 [128, n/16] (x8 groups)."""
    flat = idx_tl.reshape(-1)
    n = len(flat)
    a = np.zeros((16, n // 16), np.int16)
    a[np.arange(n) % 16, np.arange(n) // 16] = flat.astype(np.int16)
    return np.ascontiguousarray(np.tile(a, (8, 1)))


def _tile_xT(x_bf, ntiles):
    """x [M, D] bf16 -> tile-major [ntiles, D, 128] (zero padded)."""
    out = np.zeros((ntiles, D, 128), BF16)
    for t in range(ntiles):
        rows = x_bf[t * 128:(t + 1) * 128]
        if rows.shape[0]:
            out[t, :, :rows.shape[0]] = rows.T
    return out


def _perm_xT(x_bf, grids):
    """Permuted x columns for the s_dst GEMV: [BBtot, D, 128]."""
    mats = []
    for grid in grids:
        for b in range(grid.shape[0]):
            m = np.zeros((D, 128), BF16)
            gd = grid[b]
            valid = gd >= 0
            if valid.any():
                m[:, valid] = x_bf[gd[valid]].T
            mats.append(m)
    return np.stack(mats)


# ------------------------------------------------------------- device build
def _build_nc(gc):
    import concourse.bass as bass  # noqa: F401
    import concourse.mybir as mybir
    from concourse import bacc
    from concourse.tile import TileContext

    DT = mybir.dt
    Alu = mybir.AluOpType
    ActF = mybir.ActivationFunctionType
    N = gc["N"]
    SPLIT = gc["SPLIT"]
    HI_BASE = gc["HI_BASE"]
    SL_ROWS = gc["slice_rows"]
    NSLICE = gc["nslice_tiles"]
    ET = gc["etypes"]

    PH = set(os.environ.get("ATH_KERNEL_PHASES", "gemm,ag,grid,edge").split(","))
    nc = bacc.Bacc("TRN2", target_bir_lowering=False, debug=False,
                   num_devices=NCORES)
    ext = {}

    def din(name, shape, dt):
        ext[name] = nc.dram_tensor(name, shape, dt, kind="ExternalInput").ap()
        return ext[name]

    def dout(name, shape, dt):
        ext[name] = nc.dram_tensor(name, shape, dt, kind="ExternalOutput").ap()
        return ext[name]

    ident_e = din("ident", [128, 128], DT.bfloat16)
    for e in ET:
        n = e["name"]
        din(f"xT_{n}", [NSLICE, D, 128], DT.bfloat16)
        din(f"W_{n}", [D, D], DT.float32)
        din(f"a1r_{n}", [128, D], DT.float32)
        din(f"a2r_{n}", [128, D], DT.float32)
        din(f"br_{n}", [128, D], DT.float32)
        BBtot = e["BB"][0] + e["BB"][1]
        din(f"xperm_{n}", [BBtot, D, 128], DT.bfloat16)
        for h in (0, 1):
            din(f"idx{h}_{n}", [128, e["T"][h] * 8], DT.int16)
            din(f"mask{h}_{n}", [128, e["T"][h]], DT.float32)
            dout(f"hz{h}_{n}", [e["BB"][h], 128, 257], DT.bfloat16)
        e["tbl"] = nc.dram_tensor(f"tbl_{n}", [N, TCOLS], DT.bfloat16,
                                  addr_space="Shared")
        e["loc"] = nc.dram_tensor(f"loc_{n}", [SL_ROWS, TCOLS], DT.bfloat16)

    with TileContext(nc) as tc:
        with tc.tile_pool(name="const", bufs=1) as cp, \
             tc.tile_pool(name="work", bufs=2) as wp, \
             tc.tile_pool(name="gbuf", bufs=int(os.environ.get("ATH_GBUFS", "4"))) as gp, \
             tc.tile_pool(name="psA", bufs=2, space="PSUM") as psA, \
             tc.tile_pool(name="psB", bufs=2, space="PSUM") as psB, \
             tc.tile_pool(name="psE", bufs=int(os.environ.get("ATH_PSE", "4")), space="PSUM") as psE:

            ident_sb = cp.tile([128, 128], DT.bfloat16, tag="ident")
            nc.sync.dma_start(out=ident_sb[:], in_=ident_e[:])

            # -------- per-etype weight prep (W, w1 hi/lo, rhs tiles) -------
            for e in ET:
                n = e["name"]
                W_sb = cp.tile([128, 2, D], DT.float32, tag=f"W_{n}")
                nc.sync.dma_start(out=W_sb[:, 0, :], in_=ext[f"W_{n}"][0:128, :])
                nc.sync.dma_start(out=W_sb[:, 1, :], in_=ext[f"W_{n}"][128:256, :])
                a1r = cp.tile([128, D], DT.float32, tag=f"a1r_{n}")
                nc.sync.dma_start(out=a1r[:], in_=ext[f"a1r_{n}"][:])
                a2r = cp.tile([128, D], DT.float32, tag=f"a2r_{n}")
                nc.sync.dma_start(out=a2r[:], in_=ext[f"a2r_{n}"][:])
                br = cp.tile([128, D], DT.float32, tag=f"br_{n}")
                nc.sync.dma_start(out=br[:], in_=ext[f"br_{n}"][:])
                e["W_sb"], e["a1r"], e["a2r"], e["br"] = W_sb, a1r, a2r, br

            def rowdot(dst_col, Wk, arep):
                """dst_col[128,1] f32 = sum_o Wk[:,o]*arep[:,o] (per part.)."""
                tmp = wp.tile([128, D], DT.float32, tag="rdtmp")
                nc.vector.tensor_tensor(out=tmp[:], in0=Wk, in1=arep,
                                        op=Alu.mult)
                nc.vector.tensor_reduce(out=dst_col, in_=tmp[:],
                                        axis=mybir.AxisListType.X, op=Alu.add)

            def hilo(dst_bf, src_f32, width):
                """Split f32 [128,w] into bf16 hi/lo pair stored at
                dst_bf [128, w, 2] (hi at [...,0], lo at [...,1])."""
                hi_f = wp.tile([128, width], DT.float32, tag="hilo_f")
                nc.vector.tensor_copy(out=dst_bf[:, :, 0], in_=src_f32)
                nc.vector.tensor_copy(out=hi_f[:], in_=dst_bf[:, :, 0])
                lo_f = wp.tile([128, width], DT.float32, tag="hilo_l")
                nc.vector.tensor_tensor(out=lo_f[:], in0=src_f32, in1=hi_f[:],
                                        op=Alu.subtract)
                nc.vector.tensor_copy(out=dst_bf[:, :, 1], in_=lo_f[:])

            for e in ET:
                n = e["name"]
                w1f = cp.tile([128, 2], DT.float32, tag=f"w1f_{n}")
                for k in (0, 1):
                    rowdot(w1f[:, k:k + 1], e["W_sb"][:, k, :], e["a1r"][:])
                w1b = cp.tile([128, 2, 2], DT.bfloat16, tag=f"w1b_{n}")
                hilo(w1b, w1f[:], 2)
                rhs = cp.tile([128, 2, D + 2], DT.bfloat16, tag=f"rhs_{n}")
                for k in (0, 1):
                    nc.vector.tensor_copy(out=rhs[:, k, 0:D],
                                          in_=e["W_sb"][:, k, :])
                    nc.vector.tensor_copy(out=rhs[:, k, D:D + 2],
                                          in_=w1b[:, k, :])
                e["rhs"] = rhs

            # w2 / bias for the s_dst grids (uses the OTHER etype's W and b)
            for e in ET:
                n = e["name"]
                o = ET[1 - e["i"]]
                w2f = cp.tile([128, 2], DT.float32, tag=f"w2f_{n}")
                for k in (0, 1):
                    rowdot(w2f[:, k:k + 1], o["W_sb"][:, k, :], e["a2r"][:])
                w2b = cp.tile([128, 2, 2], DT.bfloat16, tag=f"w2b_{n}")
                hilo(w2b, w2f[:], 2)
                e["w2b"] = w2b
                bias = cp.tile([128, 1], DT.float32, tag=f"bias_{n}")
                t2 = wp.tile([128, 1], DT.float32, tag="biast")
                rowdot(bias[:, 0:1], o["br"][:], e["a2r"][:])
                rowdot(t2[:, 0:1], e["br"][:], e["a1r"][:])
                nc.vector.tensor_tensor(out=bias[:], in0=bias[:], in1=t2[:],
                                        op=Alu.add)
                e["bias"] = bias

            # ---------------- GEMM (own slice) + AllGather -----------------
            for e in ET:
                if "gemm" not in PH:
                    break
                n = e["name"]
                for t in range(NSLICE):
                    lhsA = wp.tile([128, 128], DT.bfloat16, tag="lhsA")
                    nc.sync.dma_start(out=lhsA[:], in_=ext[f"xT_{n}"][t, 0:128, :])
                    lhsB = wp.tile([128, 128], DT.bfloat16, tag="lhsB")
                    nc.sync.dma_start(out=lhsB[:], in_=ext[f"xT_{n}"][t, 128:256, :])
                    ps = psA.tile([128, D + 2], DT.float32, tag="gemm")
                    nc.tensor.matmul(out=ps[:], lhsT=lhsA[:], rhs=e["rhs"][:, 0, :],
                                     start=True, stop=False)
                    nc.tensor.matmul(out=ps[:], lhsT=lhsB[:], rhs=e["rhs"][:, 1, :],
                                     start=False, stop=True)
                    tt = wp.tile([128, 260], DT.bfloat16, tag="ttile")
                    nc.scalar.activation(out=tt[:, 0:D], in_=ps[:, 0:D],
                                         func=ActF.Copy)
                    ttf = tt[:].bitcast(DT.float32)
                    nc.vector.tensor_reduce(out=ttf[:, SCOL_F32:SCOL_F32 + 1],
                                            in_=ps[:, D:D + 2],
                                            axis=mybir.AxisListType.X,
                                            op=Alu.add)
                    rows = min(128, SL_ROWS - t * 128)
                    nc.sync.dma_start(
                        out=e["loc"][t * 128:t * 128 + rows, 0:258],
                        in_=tt[0:rows, 0:258])
                if "ag" not in PH:
                    continue
                nc.gpsimd.collective_compute(
                    "AllGather", Alu.bypass,
                    replica_groups=[list(range(NCORES))],
                    ins=[e["loc"][:, :].opt()],
                    outs=[e["tbl"][:, :].opt()])

            # ---------------- s_dst grids (permuted GEMV) ------------------
            for e in ET:
                n = e["name"]
                BBtot = e["BB"][0] + e["BB"][1]
                grid = cp.tile([128, BBtot], DT.float32, tag=f"grid_{n}")
                if "grid" not in PH:
                    nc.vector.memset(grid[:], 0.0)
                    e["grid"] = grid
                    continue
                for g0 in range(0, BBtot, 8):
                    g1 = min(g0 + 8, BBtot)
                    ps = psB.tile([128, 2 * (g1 - g0)], DT.float32, tag="gemv")
                    for j, b in enumerate(range(g0, g1)):
                        xpA = wp.tile([128, 128], DT.bfloat16, tag="xpA")
                        nc.sync.dma_start(out=xpA[:], in_=ext[f"xperm_{n}"][b, 0:128, :])
                        xpB = wp.tile([128, 128], DT.bfloat16, tag="xpB")
                        nc.sync.dma_start(out=xpB[:], in_=ext[f"xperm_{n}"][b, 128:256, :])
                        nc.tensor.matmul(out=ps[:, 2 * j:2 * j + 2], lhsT=xpA[:],
                                         rhs=e["w2b"][:, 0, :], start=True, stop=False)
                        nc.tensor.matmul(out=ps[:, 2 * j:2 * j + 2], lhsT=xpB[:],
                                         rhs=e["w2b"][:, 1, :], start=False, stop=True)
                    nw = g1 - g0
                    nc.vector.tensor_reduce(
                        out=grid[:, g0:g1],
                        in_=ps[:, 0:2 * nw].rearrange("p (b two) -> p b two", two=2),
                        axis=mybir.AxisListType.X, op=Alu.add)
                    nc.vector.tensor_scalar(
                        out=grid[:, g0:g1], in0=grid[:, g0:g1],
                        scalar1=e["bias"][:, 0:1], scalar2=None, op0=Alu.add)
                e["grid"] = grid

            # ---------------------- edge phase -----------------------------
            for e in ET:
                if "edge" not in PH:
                    break
                n = e["name"]
                for h in (0, 1):
                    T, BBh = e["T"][h], e["BB"][h]
                    nth = e["nt"][h]
                    in_tbl = (e["tbl"][0:SPLIT, :] if h == 0
                              else e["tbl"][HI_BASE:N, :])
                    idx_sb = cp.tile([128, T * 8], DT.int16, tag=f"idx{h}_{n}")
                    nc.sync.dma_start(out=idx_sb[:], in_=ext[f"idx{h}_{n}"][:])
                    mask_sb = cp.tile([128, T], DT.float32, tag=f"mask{h}_{n}")
                    nc.sync.dma_start(out=mask_sb[:], in_=ext[f"mask{h}_{n}"][:])
                    E_sb = cp.tile([128, T], DT.float32, tag=f"E{h}_{n}")

                    # block schedule
                    starts = np.concatenate([[0], np.cumsum(nth)]).astype(int)
                    nblk_tiles = int(starts[-1])

                    nchunk = (T + CT - 1) // CT
                    gtiles = []
                    for ci in range(nchunk):
                        c0, c1 = ci * CT, min((ci + 1) * CT, T)
                        gt = gp.tile([128, c1 - c0, TCOLS], DT.bfloat16, tag="g")
                        for k in range(c0, c1, TPC):
                            nc.gpsimd.dma_gather(
                                out_ap=gt[:, k - c0:k - c0 + TPC, :],
                                in_ap=in_tbl,
                                idxs_ap=idx_sb[:, k * 8:(k + TPC) * 8],
                                num_idxs=TPC * 128, num_idxs_reg=TPC * 128,
                                elem_size=TCOLS)
                        gtiles.append((c0, c1, gt))
                        gf = gt[:].bitcast(DT.float32)
                        # logit pre: s_src + s_dst per block-piece
                        for b in range(BBh):
                            blo = max(int(starts[b]), c0)
                            bhi = min(int(starts[b + 1]), c1)
                            if blo >= bhi:
                                continue
                            gcol = b if h == 0 else e["BB"][0] + b
                            for lo in range(blo - (blo - c0) % TPC, bhi, TPC):
                                lo2 = max(lo, blo)
                                hi = min(lo + TPC - (lo - c0) % TPC, bhi)
                                if lo2 >= hi:
                                    continue
                                nc.vector.tensor_scalar(
                                    out=E_sb[:, lo2:hi],
                                    in0=gf[:, lo2 - c0:hi - c0, SCOL_F32],
                                    scalar1=e["grid"][:, gcol:gcol + 1],
                                    scalar2=None, op0=Alu.add)
                        ce = min(c1, nblk_tiles)
                        for k0 in range(c0, ce, TPC):
                            ke = min(k0 + TPC, ce)
                            lk = wp.tile([128, TPC], DT.float32, tag="lk")
                            w = ke - k0
                            nc.vector.tensor_scalar(
                                out=lk[:, 0:w], in0=E_sb[:, k0:ke],
                                scalar1=0.01, scalar2=None, op0=Alu.mult)
                            nc.vector.tensor_tensor(
                                out=lk[:, 0:w], in0=lk[:, 0:w],
                                in1=E_sb[:, k0:ke], op=Alu.max)
                            nc.scalar.activation(out=E_sb[:, k0:ke],
                                                 in_=lk[:, 0:w], func=ActF.Exp)
                            nc.vector.tensor_tensor(
                                out=E_sb[:, k0:ke], in0=E_sb[:, k0:ke],
                                in1=mask_sb[:, k0:ke], op=Alu.mult)

                    def gslice(t):
                        for (c0, c1, gt) in gtiles:
                            if c0 <= t < c1:
                                return gt[:, t - c0, 0:D]
                        raise AssertionError

                    for b in range(BBh):
                        t0, t1 = int(starts[b]), int(starts[b + 1])
                        ps = psE.tile([128, D], DT.float32, tag="edge")
                        for t in range(t0, t1):
                            if (t - t0) % 2 == 0:
                                dg = wp.tile([128, 128], DT.bfloat16, tag="diagV")
                                nc.vector.tensor_scalar(
                                    out=dg[:], in0=ident_sb[:],
                                    scalar1=E_sb[:, t:t + 1], scalar2=None,
                                    op0=Alu.mult)
                            else:
                                dg = wp.tile([128, 128], DT.bfloat16, tag="diagA")
                                nc.scalar.activation(
                                    out=dg[:], in_=ident_sb[:], func=ActF.Copy,
                                    scale=E_sb[:, t:t + 1])
                            nc.tensor.matmul(out=ps[:], lhsT=dg[:],
                                             rhs=gslice(t),
                                             start=(t == t0), stop=(t == t1 - 1))
                        hz = wp.tile([128, 257], DT.bfloat16, tag="hz")
                        nc.scalar.activation(out=hz[:, 0:D], in_=ps[:],
                                             func=ActF.Copy)
                        # z = sum of E over the block's tiles (chunk pieces)
                        pieces = []
                        for (c0, c1, gt) in gtiles:
                            lo, hi = max(t0, c0), min(t1, c1)
                            if lo < hi:
                                pieces.append((lo, hi))
                        zf = wp.tile([128, 1], DT.float32, tag="zf")
                        nc.vector.tensor_reduce(
                            out=zf[:], in_=E_sb[:, pieces[0][0]:pieces[0][1]],
                            axis=mybir.AxisListType.X, op=Alu.add)
                        for (lo, hi) in pieces[1:]:
                            zt = wp.tile([128, 1], DT.float32, tag="zt")
                            nc.vector.tensor_reduce(
                                out=zt[:], in_=E_sb[:, lo:hi],
                                axis=mybir.AxisListType.X, op=Alu.add)
                            nc.vector.tensor_tensor(out=zf[:], in0=zf[:],
                                                    in1=zt[:], op=Alu.add)
                        nc.vector.tensor_copy(out=hz[:, D:D + 1], in_=zf[:])
                        nc.sync.dma_start(out=ext[f"hz{h}_{n}"][b, :, :],
                                          in_=hz[:])
    nc.compile()
    return nc


# ----------------------------------------------------------------- kernel()
def _prepare(inputs, cfg):
    N = cfg["N"]
    ets = [
        dict(i=0, name="t", x_src="x_drug", x_dst="x_dis", W="W_t", b="b_t",
             a="a_t", src="src_t", dst="dst_t"),
        dict(i=1, name="rt", x_src="x_dis", x_dst="x_drug", W="W_rt", b="b_rt",
             a="a_rt", src="src_rt", dst="dst_rt"),
    ]
    sts = []
    for e in ets:
        st = _build_etype(np.asarray(inputs[e["src"]]),
                          np.asarray(inputs[e["dst"]]), cfg)
        st.update(e)
        sts.append(st)

    slice_rows = N // NCORES
    nslice = (slice_rows + 127) // 128
    gc = {"N": N, "SPLIT": cfg["SPLIT"], "HI_BASE": cfg["HI_BASE"],
          "slice_rows": slice_rows, "nslice_tiles": nslice,
          "etypes": [{"name": s["name"], "i": s["i"], "BB": s["BB"],
                      "nt": s["nt"], "T": s["T"]} for s in sts]}

    in_maps = []
    core_meta = []
    ident = np.eye(128, dtype=BF16)
    xbf = {k: np.asarray(inputs[k]).astype(BF16)
           for k in ("x_drug", "x_dis")}
    for c in range(NCORES):
        im = {"ident": ident}
        meta = {}
        for s in sts:
            n = s["name"]
            arr = _core_arrays(s, c, cfg)
            meta[n] = arr
            xs = xbf[s["x_src"]]
            im[f"xT_{n}"] = _tile_xT(xs[c * slice_rows:(c + 1) * slice_rows],
                                     nslice)
            im[f"W_{n}"] = np.asarray(inputs[s["W"]], np.float32)
            a = np.asarray(inputs[s["a"]], np.float32)
            im[f"a1r_{n}"] = np.tile(a[None, :D], (128, 1))
            im[f"a2r_{n}"] = np.tile(a[None, D:], (128, 1))
            im[f"br_{n}"] = np.tile(np.asarray(inputs[s["b"]],
                                               np.float32)[None, :], (128, 1))
            im[f"xperm_{n}"] = _perm_xT(xbf[s["x_dst"]],
                                        [arr["grid0"], arr["grid1"]])
            for h in (0, 1):
                im[f"idx{h}_{n}"] = _wrap_idx(arr[f"idx{h}"])
                im[f"mask{h}_{n}"] = np.ascontiguousarray(arr[f"mask{h}"])
        in_maps.append(im)
        core_meta.append(meta)
    return gc, sts, in_maps, core_meta


def _finalize(results, sts, core_meta, inputs, cfg):
    N = cfg["N"]
    outs = {}
    for s in sts:
        n = s["name"]
        h_un = np.zeros((N, D), np.float64)
        z = np.zeros(N, np.float64)
        for c in range(NCORES):
            for h in (0, 1):
                hz = np.asarray(results[c][f"hz{h}_{n}"], np.float64)
                hz2 = hz.reshape(-1, 257)
                grid = core_meta[c][n][f"grid{h}"].reshape(-1)
                m = grid >= 0
                rows = grid[m]
                h_un[rows] += hz2[m, :D]
                z[rows] += hz2[m, D]
        b = np.asarray(inputs[s["b"]], np.float64)
        has = z > 0
        h = np.zeros((N, D), np.float32)
        h[has] = (h_un[has] / z[has, None] + b[None, :]).astype(np.float32)
        outs[n] = h
    return outs


_CACHE = {}


def kernel(**inputs):
    n_nodes = int(np.asarray(inputs["x_drug"]).shape[0])
    cfg = _default_cfg(n_nodes)
    gc, sts, in_maps, core_meta = _prepare(inputs, cfg)

    key = os.environ.get("ATH_KERNEL_PHASES", "") + os.environ.get("ATH_GBUFS", "") + os.environ.get("ATH_PSE", "") + str(
        [(e["name"], e["BB"], [int(v) for v in e["nt"][0]],
          [int(v) for v in e["nt"][1]], e["T"]) for e in gc["etypes"]])
    if key not in _CACHE:
        _CACHE[key] = _build_nc(gc)
    nc = _CACHE[key]

    if os.environ.get("ATH_KERNEL_SIM"):
        from concourse.bass_interp import MultiCoreSim
        sim = MultiCoreSim(nc, num_cores=NCORES, require_finite=False,
                           require_nnan=False)
        for c in range(NCORES):
            for k, v in in_maps[c].items():
                sim.cores[c].tensor(k)[:] = v
        sim.simulate()
        results = []
        for c in range(NCORES):
            out = {}
            for s in sts:
                for h in (0, 1):
                    name = f"hz{h}_{s['name']}"
                    out[name] = np.array(sim.cores[c].mem_tensor(name))
            results.append(out)
    else:
        from concourse.bass_utils import run_bass_kernel_spmd
        res = run_bass_kernel_spmd(nc, in_maps, core_ids=list(range(NCORES)))
        results = res.results

    outs = _finalize(results, sts, core_meta, inputs, cfg)
    return outs["rt"], outs["t"]     # (h_drug, h_dis)
